# revision 25
# baseline (speedup 1.0000x reference)
"""Trainium2 Bass kernel for nn_BondWeight (symmetric edge-weight scatter).

Problem: out[b, src[b,e]+1, dst[b,e]+1] = w[b,e] and
         out[b, dst[b,e]+1, src[b,e]+1] = w[b,e]  (set semantics, XLA-CPU
         last-write-wins order), where w = weights[bond_type], out is
         [1024, 256, 256] f32, ~1.5% nonzero.

Strategy (8 NeuronCores, data-parallel over batch, 128 batches/core):
  The output is 33.5MB/core of mostly zeros; writing it at the per-core DMA
  ceiling (~420 GB/s observed) takes ~80us, which bounds the kernel.

  - Layout: partition p holds local batch p's whole [256,256] matrix as a
    65536-value stream, so output DMA descriptors are 8KB contiguous
    (4x fewer packets than a rows-per-partition layout; 97% packet eff).
  - Values are quantized to bf16 (rel err ~1e-3 << 2e-2 tolerance) so the
    scatter engine streams half the bytes.
  - Head (stream [0,4096)): host prebuilds a DENSE bf16 image (scatter
    values folded in); it rides the input DMA and is cast to f32 by ACT
    with no GPSIMD dependency, starting the output stream ~6us earlier
    (GPSIMD pays a ~6us library IRAM load before its first scatter).
  - Tail (stream [4096,65536)): GPSIMD local_scatter builds 31 bf16
    windows of <=2046 (64KB Q7 scratch cap) in an 8-slot ring; ACT
    (scalar engine) upcasts each window into an f32 ring (16384 f32 =
    8 chunks); sync issues 32 x 1MB output DMAs (8KB descriptors).
  Engines pipeline: GPSIMD (~60us) and ACT (~60us) hide under DMA (~86us).
"""

import numpy as np

B, E, T, N = 1024, 512, 8, 256
M = 8                      # cores
BL = B // M                # 128 batches per core
NN = N * N                 # 65536
PARTS = 128                # partition p holds local batch p entirely
SLEN = NN                  # 65536: per-partition stream (f32 positions)
WIN = 2046                 # max local_scatter num_elems (64KB Q7 scratch)
DH = 4096                  # dense head length (host-built bf16)
# cast units: 2 dense head halves, then the GPSIMD scatter windows
WLEN = [WIN] * 30 + [SLEN - DH - 30 * WIN]    # scatter windows (2046*30+60)
NW = len(WLEN)             # 31
WPOS = np.concatenate([[0], np.cumsum(WLEN)]).astype(np.int64) + DH
ULEN = [DH // 2, DH // 2] + WLEN              # cast units (stream order)
UEND = np.cumsum(ULEN).astype(np.int64)       # cast unit end positions
NBUF = 8                   # bf16 ring depth (slots of WIN)
RF = 16384                 # f32 ring length per partition (f32 elems)
CH = 2048                  # ring chunk per partition = 4 batches
RC = RF // CH              # 8 ring chunks
# 32 x 1MB output DMAs (8KB contiguous descriptors); 2MB pieces with a
# 4-chunk ring were measured SLOWER (head-of-line blocking in the
# ACT-drain / DMA-gate loop dropped aggregate rate 419 -> 335 GB/s)
PIECES = [(k * CH, CH) for k in range(SLEN // CH)]

_nc_cache = {}


def _f32_to_bf16_bits(v):
    """Round-to-nearest-even f32 -> bf16, returned as int16 bit patterns."""
    bits = np.ascontiguousarray(v, dtype=np.float32).view(np.uint32)
    rnd = ((bits >> 16) & 1) + np.uint32(0x7FFF)
    return ((bits + rnd) >> 16).astype(np.uint16).view(np.int16)


def _prepare_scatter(weights, bond_src, bond_dst, bond_type):
    """Returns (comb, niw).

    comb: int16 [M, PARTS, DH+2*WTOT] = [dense bf16 image of stream
    [0,DH) | per-window scatter idx (-1 pad) | bf16 data bits].
    niw: tuple of per-window num_idxs.
    """
    w = np.ascontiguousarray(weights, dtype=np.float32)[np.asarray(bond_type)]
    s = np.asarray(bond_src, dtype=np.int64) + 1
    d = np.asarray(bond_dst, dtype=np.int64) + 1
    bb = np.arange(B, dtype=np.int64)[:, None]
    key = np.concatenate([bb * NN + s * N + d, bb * NN + d * N + s],
                         axis=1).ravel()
    order = np.tile(np.arange(2 * E, dtype=np.int64), B)
    vals = np.concatenate([w, w], axis=1).ravel()

    sortidx = np.lexsort((order, key))
    ksort = key[sortidx]
    is_last = np.empty(len(ksort), dtype=bool)
    is_last[:-1] = ksort[1:] != ksort[:-1]
    is_last[-1] = True
    sel = sortidx[is_last]            # final writer of each position
    fkey = key[sel]
    fbits = _f32_to_bf16_bits(vals[sel])

    gb = fkey // NN                   # global batch
    m = gb // BL                      # core
    p = gb % BL                       # partition == batch within core
    spos = fkey % NN                  # r*256+c: position in batch matrix

    hd = spos < DH                    # dense head entries
    dense = np.zeros((M, PARTS, DH), dtype=np.int16)
    dense[m[hd], p[hd], spos[hd]] = fbits[hd]

    tl = ~hd
    m2, p2, sp2, fb2 = m[tl], p[tl], spos[tl], fbits[tl]
    wdw = np.searchsorted(WPOS, sp2, side="right") - 1   # window index
    t = (sp2 - WPOS[wdw]).astype(np.int64)

    grp = (m2 * NW + wdw) * PARTS + p2
    o2 = np.argsort(grp, kind="stable")
    grp_s = grp[o2]
    n_ent = len(grp_s)
    new_grp = np.empty(n_ent, dtype=bool)
    new_grp[0] = True
    new_grp[1:] = grp_s[1:] != grp_s[:-1]
    gstart = np.maximum.accumulate(np.where(new_grp, np.arange(n_ent), 0))
    cc = np.arange(n_ent) - gstart

    ws, ms, ps, ts, bs = wdw[o2], m2[o2], p2[o2], t[o2], fb2[o2]

    maxcnt = np.zeros(NW, dtype=np.int64)
    np.maximum.at(maxcnt, ws, cc + 1)
    niw = np.maximum(((maxcnt + 1) // 2) * 2, 2)
    off = np.zeros(NW + 1, dtype=np.int64)
    off[1:] = np.cumsum(niw)
    wtot = int(off[-1])

    idx = np.full((M, PARTS, wtot), -1, dtype=np.int16)
    dat = np.zeros((M, PARTS, wtot), dtype=np.int16)
    col = off[ws] + cc
    idx[ms, ps, col] = ts.astype(np.int16)
    dat[ms, ps, col] = bs
    # single input image per core: [dense | idx | dat] -> one DMA
    comb = np.concatenate([dense, idx, dat], axis=2)
    return comb, tuple(int(x) for x in niw)


def _build_nc(niw):
    import concourse.bass as bass
    import concourse.mybir as mybir
    from concourse import library_config

    off = [0]
    for w_ in niw:
        off.append(off[-1] + w_)
    wtot = off[-1]

    # cast unit u -> (stream_start, length, f32-ring pieces)
    def unit_span(u):
        s0 = int(UEND[u]) - ULEN[u]
        return s0, ULEN[u]

    def ring_pieces(u):
        s0, ln = unit_span(u)
        g0 = s0 % RF
        if g0 + ln <= RF:
            return [(g0, 0, ln)]
        l1 = RF - g0
        return [(g0, 0, l1), (0, l1, ln - l1)]

    # cast units needed before stream range [0, end) is fully cast
    def uneed(end):
        return int(np.searchsorted(UEND, end - 1, side="left")) + 1

    # DMA pieces per ring chunk (for slot-drain accounting)
    npieces = [0] * (SLEN // CH)
    for s_, l_ in PIECES:
        npieces[s_ // CH] += 1

    nc = bass.Bass("TRN2", target_bir_lowering=False)
    cw = DH + 2 * wtot
    in_t = nc.dram_tensor("lsin", [PARTS, cw], mybir.dt.int16,
                          kind="ExternalInput")
    # flat f32 view of [BL, 256, 256]: partition/batch p at offset p*NN
    out_t = nc.dram_tensor("out", [BL * PARTS, 512], mybir.dt.float32,
                           kind="ExternalOutput")
    from contextlib import ExitStack
    with ExitStack() as stack:
        en = stack.enter_context
        in_sb = en(nc.sbuf_tensor("in_sb", [PARTS, cw], mybir.dt.int16))
        b16_sb = en(nc.sbuf_tensor("b16_sb", [PARTS, NBUF * WIN],
                                   mybir.dt.bfloat16))
        f32_sb = en(nc.sbuf_tensor("f32_sb", [PARTS, RF], mybir.dt.float32))
        ls_sem = en(nc.semaphore("ls_sem"))
        act_sem = en(nc.semaphore("act_sem"))
        in_sem = en(nc.semaphore("in_sem"))
        osem = [en(nc.semaphore(f"os{i}")) for i in range(RC)]
        block = en(nc.Block(no_gpsimd_drain=True))

        @block.gpsimd
        def _(gpsimd):
            gpsimd.load_library(library_config.local_scatter)
            # dummy call pays the ~6us first-use IRAM load of the library
            # while the input DMAs are still in flight; reads uninitialized
            # SBUF (scatter byte-offsets are uint16 so stay in Q7 scratch)
            gpsimd.local_scatter(
                out_ap=b16_sb[:, 0:2],
                data_ap=b16_sb[:, 4:6],
                idxs_ap=b16_sb[:, 8:10].bitcast(mybir.dt.int16),
                channels=PARTS, num_elems=2, num_idxs=2)
            for w in range(NW):
                if w == 0:
                    gpsimd.wait_ge(in_sem, 16)
                if w >= NBUF:
                    # bf16 slot reuse: ACT consumed window w-NBUF
                    # (its cast is unit w-NBUF+2)
                    gpsimd.wait_ge(act_sem, w - NBUF + 3)
                kb = (w % NBUF) * WIN
                gpsimd.local_scatter(
                    out_ap=b16_sb[:, kb:kb + WLEN[w]],
                    data_ap=in_sb[:, DH + wtot + off[w]:
                                  DH + wtot + off[w + 1]]
                        .bitcast(mybir.dt.bfloat16),
                    idxs_ap=in_sb[:, DH + off[w]:DH + off[w + 1]],
                    channels=PARTS,
                    num_elems=WLEN[w],
                    num_idxs=niw[w],
                ).then_inc(ls_sem, 1)

        @block.scalar
        def _(scalar):
            # ONE input DMA (dense|idx|dat, ~1.8MB, 14KB descriptors) --
            # each dma_start costs ~620ns of issuing-engine time and
            # parallel small input DMAs only reached ~255 GB/s; then a
            # 2-element dummy cast pre-warms the ACT table (~1.3us load)
            # while the input data is in flight
            scalar.dma_start(in_sb[:], in_t[:]).then_inc(in_sem, 16)
            scalar.copy(f32_sb[:, 0:2], b16_sb[:, 16:18])
            drained = set()
            for u in range(len(ULEN)):
                if u < 2:
                    scalar.wait_ge(in_sem, 16)
                else:
                    scalar.wait_ge(ls_sem, u - 1)
                s0, ln = unit_span(u)
                for cx in range(max(s0 // CH, RC),
                                (s0 + ln - 1) // CH + 1):
                    if cx not in drained:
                        drained.add(cx)
                        # pieces already drained on this slot before cx
                        prior = sum(npieces[c_] for c_ in
                                    range(cx % RC, cx, RC))
                        scalar.wait_ge(osem[cx % RC], 16 * prior)
                pieces = ring_pieces(u)
                for i, (g0, sfo, ln_) in enumerate(pieces):
                    if u < 2:
                        src = in_sb[:, u * (DH // 2) + sfo:
                                    u * (DH // 2) + sfo + ln_] \
                            .bitcast(mybir.dt.bfloat16)
                    else:
                        kb = ((u - 2) % NBUF) * WIN
                        src = b16_sb[:, kb + sfo:kb + sfo + ln_]
                    ins = scalar.copy(f32_sb[:, g0:g0 + ln_], src)
                    if i == len(pieces) - 1:
                        ins.then_inc(act_sem, 1)

        @block.sync
        def _(sync):
            for s_, l_ in PIECES:
                sync.wait_ge(act_sem, uneed(s_ + l_))
                # partition p = local batch p: contiguous l_*4B descriptors
                ap = bass.AP(out_t, s_, [[NN, PARTS], [1, l_]])
                rs = s_ % RF
                sync.dma_start(ap, f32_sb[:, rs:rs + l_]) \
                    .then_inc(osem[(s_ // CH) % RC], 16)
            for sl in range(RC):
                tot = sum(npieces[c_] for c_ in range(sl, SLEN // CH, RC))
                sync.wait_ge(osem[sl], 16 * tot)

    from concourse.library_overlay import lower_extended_insts
    lower_extended_insts(nc)
    return nc


def _get_nc(niw):
    if niw not in _nc_cache:
        _nc_cache[niw] = _build_nc(niw)
    return _nc_cache[niw]


def run_with_stats(inputs, trace=False):
    """Run the kernel; returns (output [B,N,N] f32, exec_time_ns or None)."""
    from concourse.bass_utils import run_bass_kernel_spmd

    comb, niw = _prepare_scatter(
        inputs["weights"], inputs["bond_src"],
        inputs["bond_dst"], inputs["bond_type"])
    nc = _get_nc(niw)
    in_maps = [{"lsin": np.ascontiguousarray(comb[m])} for m in range(M)]
    res = run_bass_kernel_spmd(nc, in_maps, core_ids=list(range(M)),
                               trace=trace)
    out = np.empty((B, N, N), dtype=np.float32)
    for m in range(M):
        o = res.results[m]["out"]            # f32 [BL*PARTS, 512]
        out[m * BL:(m + 1) * BL] = np.asarray(o).reshape(BL, N, N)
    return out, res.exec_time_ns


def kernel(weights, bond_src, bond_dst, bond_type, num_nodes):
    assert int(num_nodes) == N
    out, _ = run_with_stats({
        "weights": np.asarray(weights),
        "bond_src": np.asarray(bond_src),
        "bond_dst": np.asarray(bond_dst),
        "bond_type": np.asarray(bond_type),
    })
    return out


# revision 26
# speedup vs baseline: 1.1730x; 1.1730x over previous
"""Trainium2 Bass kernel for nn_BondWeight (symmetric edge-weight scatter).

Problem: out[b, src[b,e]+1, dst[b,e]+1] = w[b,e] and
         out[b, dst[b,e]+1, src[b,e]+1] = w[b,e]  (set semantics, XLA-CPU
         last-write-wins order), where w = weights[bond_type], out is
         [1024, 256, 256] f32, ~1.5% nonzero.

Strategy (8 NeuronCores, data-parallel over batch, 128 batches/core):
  The output is 33.5MB/core of mostly zeros; writing it at the per-core DMA
  ceiling (~420 GB/s observed) takes ~80us, which bounds the kernel.

  - Layout: partition p holds local batch p's whole [256,256] matrix as a
    65536-value stream, so output DMA descriptors are 8KB contiguous
    (4x fewer packets than a rows-per-partition layout; 97% packet eff).
  - Values are quantized to bf16 (rel err ~1e-3 << 2e-2 tolerance) so the
    scatter engine streams half the bytes.
  - Head (stream [0,4096)): host prebuilds a DENSE bf16 image (scatter
    values folded in); it rides the input DMA and is cast to f32 by ACT
    with no GPSIMD dependency, starting the output stream ~6us earlier
    (GPSIMD pays a ~6us library IRAM load before its first scatter).
  - Tail (stream [4096,65536)): GPSIMD local_scatter builds 31 bf16
    windows of <=2046 (64KB Q7 scratch cap) in an 8-slot ring; ACT
    (scalar engine) upcasts each window into an f32 ring (16384 f32 =
    8 chunks); sync issues 32 x 1MB output DMAs (8KB descriptors).
  Engines pipeline: GPSIMD (~60us) and ACT (~60us) hide under DMA (~86us).
"""

import numpy as np

B, E, T, N = 1024, 512, 8, 256
M = 8                      # cores
BL = B // M                # 128 batches per core
NN = N * N                 # 65536
PARTS = 128                # partition p holds local batch p entirely
SLEN = NN                  # 65536: per-partition stream (f32 positions)
WIN = 2046                 # max local_scatter num_elems (64KB Q7 scratch)
DH = 4096                  # dense head length (host-built bf16)
# cast units: 2 dense head halves, then the GPSIMD scatter windows
WLEN = [WIN] * 30 + [SLEN - DH - 30 * WIN]    # scatter windows (2046*30+60)
NW = len(WLEN)             # 31
WPOS = np.concatenate([[0], np.cumsum(WLEN)]).astype(np.int64) + DH
ULEN = [DH // 2, DH // 2] + WLEN              # cast units (stream order)
UEND = np.cumsum(ULEN).astype(np.int64)       # cast unit end positions
NBUF = 8                   # bf16 ring depth (slots of WIN)
RF = 16384                 # f32 ring length per partition (f32 elems)
CH = 2048                  # ring chunk per partition = 4 batches
RC = RF // CH              # 8 ring chunks
# 32 x 1MB output DMAs (8KB contiguous descriptors); 2MB pieces with a
# 4-chunk ring were measured SLOWER (head-of-line blocking in the
# ACT-drain / DMA-gate loop dropped aggregate rate 419 -> 335 GB/s)
PIECES = [(k * CH, CH) for k in range(SLEN // CH)]

_nc_cache = {}


def _f32_to_bf16_bits(v):
    """Round-to-nearest-even f32 -> bf16, returned as int16 bit patterns."""
    bits = np.ascontiguousarray(v, dtype=np.float32).view(np.uint32)
    rnd = ((bits >> 16) & 1) + np.uint32(0x7FFF)
    return ((bits + rnd) >> 16).astype(np.uint16).view(np.int16)


def _prepare_scatter(weights, bond_src, bond_dst, bond_type):
    """Returns (dense, idx, dat, niw).

    dense: int16 [M, PARTS, DH] bf16 image of stream [0, DH).
    idx/dat: int16 [M, PARTS, WTOT] per-window scatter slots (idx==-1 pad);
    dat holds bf16 bit patterns. niw: tuple of per-window num_idxs.
    """
    w = np.ascontiguousarray(weights, dtype=np.float32)[np.asarray(bond_type)]
    s = np.asarray(bond_src, dtype=np.int64) + 1
    d = np.asarray(bond_dst, dtype=np.int64) + 1
    bb = np.arange(B, dtype=np.int64)[:, None]
    key = np.concatenate([bb * NN + s * N + d, bb * NN + d * N + s],
                         axis=1).ravel()
    order = np.tile(np.arange(2 * E, dtype=np.int64), B)
    vals = np.concatenate([w, w], axis=1).ravel()

    sortidx = np.lexsort((order, key))
    ksort = key[sortidx]
    is_last = np.empty(len(ksort), dtype=bool)
    is_last[:-1] = ksort[1:] != ksort[:-1]
    is_last[-1] = True
    sel = sortidx[is_last]            # final writer of each position
    fkey = key[sel]
    fbits = _f32_to_bf16_bits(vals[sel])

    gb = fkey // NN                   # global batch
    m = gb // BL                      # core
    p = gb % BL                       # partition == batch within core
    spos = fkey % NN                  # r*256+c: position in batch matrix

    hd = spos < DH                    # dense head entries
    dense = np.zeros((M, PARTS, DH), dtype=np.int16)
    dense[m[hd], p[hd], spos[hd]] = fbits[hd]

    tl = ~hd
    m2, p2, sp2, fb2 = m[tl], p[tl], spos[tl], fbits[tl]
    wdw = np.searchsorted(WPOS, sp2, side="right") - 1   # window index
    t = (sp2 - WPOS[wdw]).astype(np.int64)

    grp = (m2 * NW + wdw) * PARTS + p2
    o2 = np.argsort(grp, kind="stable")
    grp_s = grp[o2]
    n_ent = len(grp_s)
    new_grp = np.empty(n_ent, dtype=bool)
    new_grp[0] = True
    new_grp[1:] = grp_s[1:] != grp_s[:-1]
    gstart = np.maximum.accumulate(np.where(new_grp, np.arange(n_ent), 0))
    cc = np.arange(n_ent) - gstart

    ws, ms, ps, ts, bs = wdw[o2], m2[o2], p2[o2], t[o2], fb2[o2]

    maxcnt = np.zeros(NW, dtype=np.int64)
    np.maximum.at(maxcnt, ws, cc + 1)
    niw = np.maximum(((maxcnt + 1) // 2) * 2, 2)
    off = np.zeros(NW + 1, dtype=np.int64)
    off[1:] = np.cumsum(niw)
    wtot = int(off[-1])

    idx = np.full((M, PARTS, wtot), -1, dtype=np.int16)
    dat = np.zeros((M, PARTS, wtot), dtype=np.int16)
    col = off[ws] + cc
    idx[ms, ps, col] = ts.astype(np.int16)
    dat[ms, ps, col] = bs
    return dense, idx, dat, tuple(int(x) for x in niw)


def _build_nc(niw):
    import concourse.bass as bass
    import concourse.mybir as mybir
    from concourse import library_config

    off = [0]
    for w_ in niw:
        off.append(off[-1] + w_)
    wtot = off[-1]

    # cast unit u -> (stream_start, length, f32-ring pieces)
    def unit_span(u):
        s0 = int(UEND[u]) - ULEN[u]
        return s0, ULEN[u]

    def ring_pieces(u):
        s0, ln = unit_span(u)
        g0 = s0 % RF
        if g0 + ln <= RF:
            return [(g0, 0, ln)]
        l1 = RF - g0
        return [(g0, 0, l1), (0, l1, ln - l1)]

    # cast units needed before stream range [0, end) is fully cast
    def uneed(end):
        return int(np.searchsorted(UEND, end - 1, side="left")) + 1

    # DMA pieces per ring chunk (for slot-drain accounting)
    npieces = [0] * (SLEN // CH)
    for s_, l_ in PIECES:
        npieces[s_ // CH] += 1

    nc = bass.Bass("TRN2", target_bir_lowering=False)
    dn_t = nc.dram_tensor("lsdense", [PARTS, DH], mybir.dt.int16,
                          kind="ExternalInput")
    idx_t = nc.dram_tensor("lsidx", [PARTS, wtot], mybir.dt.int16,
                           kind="ExternalInput")
    dat_t = nc.dram_tensor("lsdat", [PARTS, wtot], mybir.dt.int16,
                           kind="ExternalInput")
    # flat f32 view of [BL, 256, 256]: partition/batch p at offset p*NN
    out_t = nc.dram_tensor("out", [BL * PARTS, 512], mybir.dt.float32,
                           kind="ExternalOutput")
    from contextlib import ExitStack
    with ExitStack() as stack:
        en = stack.enter_context
        dn_sb = en(nc.sbuf_tensor("dn_sb", [PARTS, DH], mybir.dt.int16))
        idx_sb = en(nc.sbuf_tensor("idx_sb", [PARTS, wtot], mybir.dt.int16))
        dat_sb = en(nc.sbuf_tensor("dat_sb", [PARTS, wtot], mybir.dt.int16))
        b16_sb = en(nc.sbuf_tensor("b16_sb", [PARTS, NBUF * WIN],
                                   mybir.dt.bfloat16))
        f32_sb = en(nc.sbuf_tensor("f32_sb", [PARTS, RF], mybir.dt.float32))
        ls_sem = en(nc.semaphore("ls_sem"))
        act_sem = en(nc.semaphore("act_sem"))
        dn_sems = [en(nc.semaphore(f"dn{i}")) for i in range(2)]
        ch_sems = [en(nc.semaphore(f"ch{i}")) for i in range(4)]
        osem = [en(nc.semaphore(f"os{i}")) for i in range(RC)]
        block = en(nc.Block(no_gpsimd_drain=True))

        @block.gpsimd
        def _(gpsimd):
            gpsimd.load_library(library_config.local_scatter)
            # dummy call pays the ~6us first-use IRAM load of the library
            # while the input DMAs are still in flight; reads uninitialized
            # SBUF (scatter byte-offsets are uint16 so stay in Q7 scratch)
            gpsimd.local_scatter(
                out_ap=b16_sb[:, 0:2],
                data_ap=b16_sb[:, 4:6],
                idxs_ap=b16_sb[:, 8:10].bitcast(mybir.dt.int16),
                channels=PARTS, num_elems=2, num_idxs=2)
            for w in range(NW):
                if w == 0:
                    gpsimd.wait_ge(ch_sems[0], 16)
                    gpsimd.wait_ge(ch_sems[1], 16)
                if w >= NBUF:
                    # bf16 slot reuse: ACT consumed window w-NBUF
                    # (its cast is unit w-NBUF+2)
                    gpsimd.wait_ge(act_sem, w - NBUF + 3)
                kb = (w % NBUF) * WIN
                gpsimd.local_scatter(
                    out_ap=b16_sb[:, kb:kb + WLEN[w]],
                    data_ap=dat_sb[:, off[w]:off[w + 1]]
                        .bitcast(mybir.dt.bfloat16),
                    idxs_ap=idx_sb[:, off[w]:off[w + 1]],
                    channels=PARTS,
                    num_elems=WLEN[w],
                    num_idxs=niw[w],
                ).then_inc(ls_sem, 1)

        @block.scalar
        def _(scalar):
            # only the 2 dense-head DMAs are issued here (each dma_start
            # costs ~620ns of engine time; idx/dat issue from sync), then
            # a 2-element dummy cast pre-warms the ACT table (~1.3us load)
            # while the dense data is in flight
            for h in range(2):
                hs = slice(h * (DH // 2), (h + 1) * (DH // 2))
                scalar.dma_start(dn_sb[:, hs], dn_t[:, hs]) \
                    .then_inc(dn_sems[h], 16)
            scalar.copy(f32_sb[:, 0:2], b16_sb[:, 16:18])
            drained = set()
            for u in range(len(ULEN)):
                if u < 2:
                    scalar.wait_ge(dn_sems[u], 16)
                else:
                    scalar.wait_ge(ls_sem, u - 1)
                s0, ln = unit_span(u)
                for cx in range(max(s0 // CH, RC),
                                (s0 + ln - 1) // CH + 1):
                    if cx not in drained:
                        drained.add(cx)
                        # pieces already drained on this slot before cx
                        prior = sum(npieces[c_] for c_ in
                                    range(cx % RC, cx, RC))
                        scalar.wait_ge(osem[cx % RC], 16 * prior)
                pieces = ring_pieces(u)
                for i, (g0, sfo, ln_) in enumerate(pieces):
                    if u < 2:
                        src = dn_sb[:, u * (DH // 2) + sfo:
                                    u * (DH // 2) + sfo + ln_] \
                            .bitcast(mybir.dt.bfloat16)
                    else:
                        kb = ((u - 2) % NBUF) * WIN
                        src = b16_sb[:, kb + sfo:kb + sfo + ln_]
                    ins = scalar.copy(f32_sb[:, g0:g0 + ln_], src)
                    if i == len(pieces) - 1:
                        ins.then_inc(act_sem, 1)

        @block.sync
        def _(sync):
            # idx/dat ship as two whole-tensor DMAs; they land (~11.3us)
            # before GPSIMD's library load completes (~12.8us)
            sync.dma_start(idx_sb[:], idx_t[:]).then_inc(ch_sems[0], 16)
            sync.dma_start(dat_sb[:], dat_t[:]).then_inc(ch_sems[1], 16)
            for s_, l_ in PIECES:
                sync.wait_ge(act_sem, uneed(s_ + l_))
                # partition p = local batch p: contiguous l_*4B descriptors
                ap = bass.AP(out_t, s_, [[NN, PARTS], [1, l_]])
                rs = s_ % RF
                sync.dma_start(ap, f32_sb[:, rs:rs + l_]) \
                    .then_inc(osem[(s_ // CH) % RC], 16)
            for sl in range(RC):
                tot = sum(npieces[c_] for c_ in range(sl, SLEN // CH, RC))
                sync.wait_ge(osem[sl], 16 * tot)

    from concourse.library_overlay import lower_extended_insts
    lower_extended_insts(nc)
    return nc


def _get_nc(niw):
    if niw not in _nc_cache:
        _nc_cache[niw] = _build_nc(niw)
    return _nc_cache[niw]


def run_with_stats(inputs, trace=False):
    """Run the kernel; returns (output [B,N,N] f32, exec_time_ns or None)."""
    from concourse.bass_utils import run_bass_kernel_spmd

    dense, idx, dat, niw = _prepare_scatter(
        inputs["weights"], inputs["bond_src"],
        inputs["bond_dst"], inputs["bond_type"])
    nc = _get_nc(niw)
    in_maps = [{"lsdense": np.ascontiguousarray(dense[m]),
                "lsidx": np.ascontiguousarray(idx[m]),
                "lsdat": np.ascontiguousarray(dat[m])} for m in range(M)]
    res = run_bass_kernel_spmd(nc, in_maps, core_ids=list(range(M)),
                               trace=trace)
    out = np.empty((B, N, N), dtype=np.float32)
    for m in range(M):
        o = res.results[m]["out"]            # f32 [BL*PARTS, 512]
        out[m * BL:(m + 1) * BL] = np.asarray(o).reshape(BL, N, N)
    return out, res.exec_time_ns


def kernel(weights, bond_src, bond_dst, bond_type, num_nodes):
    assert int(num_nodes) == N
    out, _ = run_with_stats({
        "weights": np.asarray(weights),
        "bond_src": np.asarray(bond_src),
        "bond_dst": np.asarray(bond_dst),
        "bond_type": np.asarray(bond_type),
    })
    return out
